# revision 1
# baseline (speedup 1.0000x reference)
"""Mixture-of-Softmaxes kernel for 8 Trainium2 NeuronCores.

Strategy: tensor-parallel over the vocab dimension (V=100000 -> 12500/core).
Each core computes all B rows for its vocab shard: per-head logits via bf16
matmuls, exp via ScalarE (with row-sum side-accumulation), a tiny [128,1]
per-head AllReduce of the softmax denominators across cores, then the
pi-weighted mixture on VectorE. Output is gathered on the host by
concatenating the vocab shards (bf16 -> f32 cast on host).

Pipelining: exp values live in a ring of half-head tiles with a spare
slot, so ScalarE/TensorE stream into the next block while the current
block's mixture waits on its collectives. Mixture passes are gated
per-head so collective latency overlaps the remaining heads' compute.
projT is spilled to DRAM and per-block weight slices are prefetched a
block ahead to free SBUF for the ring.

Host-side prep: inputs are transposed (contraction dim -> SBUF partitions)
and cast to bf16 before DMA, so the kernel needs no on-chip transposes.
"""

import numpy as np
import ml_dtypes

import concourse.bass as bass
import concourse.mybir as mybir
import concourse.tile as tile
from concourse import bacc
from concourse.bass_utils import run_bass_kernel_spmd
from concourse.bass_interp import get_hw_module

B, H, D, V = 1024, 4, 256, 100000
N_CORES = 8
V_S = V // N_CORES          # 12500 vocab entries per core
KT = D // 128               # 2 contraction k-tiles
BBLK = 128                  # b rows per block (= SBUF partitions)
N_BBLK = B // BBLK          # 8 blocks
HALF = V_S // 2             # 6250: e-ring slot width
QRT = V_S // 4              # 3125: mixture/acc chunk width
E_SLOTS = 9                 # 8 per block + 1 slack (ScalarE runs ahead)

# psum chunking within a half: matmul N<=512 (one bank), ACT reads 4 banks
_HCHUNKS = [(0, 2048), (2048, 2048), (4096, 2048), (6144, 106)]

F32 = mybir.dt.float32
BF16 = mybir.dt.bfloat16

_RUN_KWARGS = {}  # test harness may set trace/tmpdir here
_CACHE = {}


def _build():
    nc = bacc.Bacc("TRN2", target_bir_lowering=False, debug=False,
                   num_devices=N_CORES)
    xT = nc.dram_tensor("xT", [D, B], BF16, kind="ExternalInput").ap()
    pmT = nc.dram_tensor("pmT", [D, H * D], BF16, kind="ExternalInput").ap()
    mmT = nc.dram_tensor("mmT", [D, H], BF16, kind="ExternalInput").ap()
    embT = nc.dram_tensor("embT", [D, V_S], BF16, kind="ExternalInput").ap()
    out = nc.dram_tensor("out", [B, V_S], BF16, kind="ExternalOutput").ap()

    with tile.TileContext(nc) as tc:
        _body(tc, xT, pmT, mmT, embT, out)
        tc._pool_ctx.close()

    nc.compile()
    nc.m = get_hw_module(nc.m)
    return nc


def _body(tc, xT, pmT, mmT, embT, out):
    nc = tc.nc
    Exp = mybir.ActivationFunctionType.Exp
    Tanh = mybir.ActivationFunctionType.Tanh
    add = mybir.AluOpType.add

    import contextlib
    ctx = contextlib.ExitStack()
    tc._pool_ctx = ctx
    singles = ctx.enter_context(tc.tile_pool(name="singles", bufs=1))
    work = ctx.enter_context(tc.tile_pool(name="work", bufs=3))
    mix = ctx.enter_context(tc.tile_pool(name="mix", bufs=2))
    lwp = ctx.enter_context(tc.tile_pool(name="lwp", bufs=2))
    ering = ctx.enter_context(tc.tile_pool(name="ering", bufs=E_SLOTS))
    psum = ctx.enter_context(tc.tile_pool(name="psum", bufs=2, space="PSUM"))
    dram = ctx.enter_context(tc.tile_pool(name="dram", bufs=4, space="DRAM"))

    # ---- resident SBUF inputs (xT/pmT borrow e-ring slots: prologue-only)
    sb_xT, sb_pmT, sb_mmT, sb_embT = [], [], [], []
    for k in range(KT):
        t = ering.tile([128, HALF], BF16, tag="e", name=f"xT{k}")
        nc.sync.dma_start(out=t[:, :B], in_=xT[k * 128:(k + 1) * 128, :])
        sb_xT.append(t[:, :B])
        t = ering.tile([128, HALF], BF16, tag="e", name=f"pmT{k}")
        nc.sync.dma_start(out=t[:, :H * D], in_=pmT[k * 128:(k + 1) * 128, :])
        sb_pmT.append(t[:, :H * D])
        t = work.tile([128, H], BF16, tag=f"mmT{k}", name=f"mmT{k}")
        nc.sync.dma_start(out=t, in_=mmT[k * 128:(k + 1) * 128, :])
        sb_mmT.append(t)
        t = singles.tile([128, V_S], BF16, tag=f"embT{k}", name=f"embT{k}")
        nc.sync.dma_start(out=t, in_=embT[k * 128:(k + 1) * 128, :])
        sb_embT.append(t)

    # ---- projT[h][kd] = tanh(proj_mat_h @ x.T), spilled to DRAM ----
    projT_dram = [[dram.tile([128, B], BF16, tag=f"pjd{h}_{kd}", bufs=1,
                             name=f"pjd{h}_{kd}")
                   for kd in range(KT)] for h in range(H)]
    for h in range(H):
        for kd in range(KT):
            for bs in range(B // 512):
                ps = psum.tile([128, 2048], F32, tag="ps", name="ps")
                for kc in range(KT):
                    nc.tensor.matmul(
                        ps[:, :512],
                        sb_pmT[kc][:, h * D + kd * 128: h * D + (kd + 1) * 128],
                        sb_xT[kc][:, bs * 512:(bs + 1) * 512],
                        start=(kc == 0), stop=(kc == KT - 1),
                    )
                stg = work.tile([128, 512], BF16, tag="stg", name="stg")
                nc.scalar.activation(out=stg, in_=ps[:, :512], func=Tanh)
                nc.sync.dma_start(
                    out=projT_dram[h][kd][:, bs * 512:(bs + 1) * 512],
                    in_=stg)

    # ---- pi[b, h] = softmax_h(x @ mix_mat.T) per b-block ----
    sb_pi = []
    for i in range(N_BBLK):
        ps = psum.tile([128, 2048], F32, tag="ps", name="ps")
        for kc in range(KT):
            nc.tensor.matmul(
                ps[:, :H],
                sb_xT[kc][:, i * 128:(i + 1) * 128],
                sb_mmT[kc],
                start=(kc == 0), stop=(kc == KT - 1),
            )
        m = work.tile([128, 1], F32, tag="pim", name="pim")
        nc.vector.tensor_reduce(out=m, in_=ps[:, :H],
                                axis=mybir.AxisListType.X,
                                op=mybir.AluOpType.max)
        negm = work.tile([128, 1], F32, tag="pinegm", name="pinegm")
        nc.vector.tensor_scalar_mul(negm, m, -1.0)
        e = work.tile([128, H], F32, tag="pie", name="pie")
        nc.scalar.activation(out=e, in_=ps[:, :H], func=Exp, bias=negm)
        s = work.tile([128, 1], F32, tag="pis", name="pis")
        nc.vector.tensor_reduce(out=s, in_=e, axis=mybir.AxisListType.X,
                                op=add)
        rs = work.tile([128, 1], F32, tag="pirs", name="pirs")
        nc.vector.reciprocal(rs, s)
        pi = singles.tile([128, H], F32, tag=f"pi{i}", name=f"pi{i}")
        nc.vector.tensor_scalar_mul(pi, e, rs)
        sb_pi.append(pi)

    # ---- main loop over b-blocks ----
    def load_weights(i):
        lw = {}
        for h in range(H):
            for kc in range(KT):
                t = lwp.tile([128, 128], BF16, tag=f"lw{h}_{kc}",
                             name=f"lw{h}_{kc}")
                nc.sync.dma_start(
                    out=t, in_=projT_dram[h][kc][:, i * 128:(i + 1) * 128])
                lw[(h, kc)] = t
        return lw

    lw_cur = load_weights(0)
    for i in range(N_BBLK):
        accs = [mix.tile([128, QRT], BF16, tag="acc", bufs=4, name=f"acc{q}")
                for q in range(4)]
        lw_next = None
        for h in range(H):
            sparts = work.tile([128, 8], F32, tag=f"sp{h}", name=f"sp{h}")
            ehalves = []
            for half in range(2):
                ehalf = ering.tile([128, HALF], BF16, tag="e",
                                   name=f"e{h}_{half}")
                ehalves.append(ehalf)
                for ci, (c0, cw) in enumerate(_HCHUNKS):
                    v0 = half * HALF + c0
                    ps = psum.tile([128, 2048], F32, tag="ps", name="ps")
                    for kc in range(KT):
                        for ns in range((cw + 511) // 512):
                            n0 = ns * 512
                            nw = min(512, cw - n0)
                            nc.tensor.matmul(
                                ps[:, n0:n0 + nw],
                                lw_cur[(h, kc)],
                                sb_embT[kc][:, v0 + n0:v0 + n0 + nw],
                                start=(kc == 0), stop=(kc == KT - 1),
                            )
                    if ci < 3:
                        nc.scalar.activation(
                            out=ehalf[:, c0:c0 + cw], in_=ps[:, :cw],
                            func=Exp,
                            accum_out=sparts[:, half * 4 + ci:
                                             half * 4 + ci + 1],
                        )
                    else:
                        # tail chunk: skip ScalarE's accum register read;
                        # the 106-wide row-sum goes to DVE (has slack)
                        nc.scalar.activation(
                            out=ehalf[:, c0:c0 + cw], in_=ps[:, :cw],
                            func=Exp)
                        nc.vector.tensor_reduce(
                            out=sparts[:, half * 4 + 3:half * 4 + 4],
                            in_=ehalf[:, c0:c0 + cw],
                            axis=mybir.AxisListType.X, op=add)
            if h == 0 and i + 1 < N_BBLK:
                # prefetch next block's weight slices during head 1
                lw_next = load_weights(i + 1)

            # head-h denominator -> AllReduce across vocab shards
            s_loc = work.tile([128, 1], F32, tag=f"sloc{h}", name=f"sloc{h}")
            nc.vector.tensor_reduce(
                out=s_loc, in_=sparts,
                axis=mybir.AxisListType.X, op=add)
            cc_in = dram.tile([128, 1], F32, tag=f"ccin{h}", name=f"ccin{h}")
            cc_out = dram.tile([128, 1], F32, tag=f"ccout{h}",
                               name=f"ccout{h}")
            nc.gpsimd.dma_start(out=cc_in[:], in_=s_loc)
            nc.gpsimd.collective_compute(
                "AllReduce", add,
                replica_groups=[list(range(N_CORES))],
                ins=[cc_in.opt()], outs=[cc_out.opt()],
            )
            s_glob = work.tile([128, 1], F32, tag=f"sglob{h}",
                               name=f"sglob{h}")
            # gpsimd queue, NOT sync: the sync FIFO carries the big
            # output DMAs whose sem-waits would head-of-line-block this
            # latency-critical read (measured: sync placement costs ~40us)
            nc.gpsimd.dma_start(out=s_glob, in_=cc_out[:])
            rS = work.tile([128, 1], F32, tag=f"rS{h}", name=f"rS{h}")
            nc.vector.reciprocal(rS, s_glob)
            w = work.tile([128, 1], F32, tag=f"w{h}", name=f"w{h}")
            nc.vector.tensor_mul(w, sb_pi[i][:, h:h + 1], rS)

            # mixture pass h (DVE): tensor_scalar at 4x bf16, adds at 2x
            for q in range(4):
                half, sub = divmod(q, 2)
                esl = ehalves[half][:, sub * QRT:(sub + 1) * QRT]
                if h == 0:
                    nc.vector.tensor_scalar_mul(accs[q], esl, w)
                else:
                    t1 = mix.tile([128, QRT], BF16, tag="t1", name="t1")
                    nc.vector.tensor_scalar_mul(t1, esl, w)
                    nc.vector.tensor_tensor(
                        out=accs[q], in0=accs[q], in1=t1, op=add)
                if h == H - 1:
                    nc.sync.dma_start(
                        out=out[i * 128:(i + 1) * 128,
                                q * QRT:(q + 1) * QRT],
                        in_=accs[q])
        if lw_next is not None:
            lw_cur = lw_next


def _get_nc():
    if "nc" not in _CACHE:
        _CACHE["nc"] = _build()
    return _CACHE["nc"]


def kernel(x, proj_mat, mix_mat, emb):
    nc = _get_nc()
    bf = ml_dtypes.bfloat16
    xT = np.ascontiguousarray(x.astype(bf).T)
    pmT = np.ascontiguousarray(proj_mat.astype(bf).T)
    mmT = np.ascontiguousarray(mix_mat.astype(bf).T)
    emb_bf = emb.astype(bf)
    in_maps = []
    for c in range(N_CORES):
        embT = np.ascontiguousarray(emb_bf[c * V_S:(c + 1) * V_S].T)
        in_maps.append({"xT": xT, "pmT": pmT, "mmT": mmT, "embT": embT})
    res = run_bass_kernel_spmd(nc, in_maps, list(range(N_CORES)),
                               **_RUN_KWARGS)
    _CACHE["last_result"] = res
    return np.concatenate(
        [res.results[c]["out"].astype(np.float32) for c in range(N_CORES)],
        axis=1)



# revision 5
# speedup vs baseline: 1.2153x; 1.2153x over previous
"""Mixture-of-Softmaxes kernel for 8 Trainium2 NeuronCores.

Strategy: tensor-parallel over the vocab dimension (V=100000 -> 12500/core).
Each core computes all B rows for its vocab shard: per-head logits via bf16
matmuls, exp via ScalarE (with row-sum side-accumulation), ONE per-block
[128,4] AllReduce of all four heads' softmax denominators (8 collectives
total instead of 32 -- the CC engine's ~20us/op service time was the v1
pacer), then a pi-weighted mixture on VectorE that lags production by one
block. Output is gathered on the host by concatenating the vocab shards.

Mixture is fused: scalar_tensor_tensor accumulates (e_h * w_h) + acc
in-place into the LAST head's e-tile (freed last by ring order), so no
separate accumulator SBUF is needed and the e-ring deepens to 11 slots.
Halves are asymmetric (6144 = 3 full psum chunks, 6356 = 3 chunks + 212
tail) so only one tail activation per head is paid instead of two.

Host-side prep: inputs are transposed (contraction dim -> SBUF partitions)
and cast to bf16 before DMA, so the kernel needs no on-chip transposes.
"""

import numpy as np
import ml_dtypes

import concourse.bass as bass
import concourse.mybir as mybir
import concourse.tile as tile
from concourse import bacc
from concourse.bass_utils import run_bass_kernel_spmd
from concourse.bass_interp import get_hw_module

B, H, D, V = 1024, 4, 256, 100000
N_CORES = 8
V_S = V // N_CORES          # 12500 vocab entries per core
KT = D // 128               # 2 contraction k-tiles
BBLK = 128                  # b rows per block (= SBUF partitions)
N_BBLK = B // BBLK          # 8 blocks
HALF_A = 6144               # 3 psum chunks, no tail
HALF_B = V_S - HALF_A       # 6356 = 3 psum chunks + 212 tail
E_SLOTS = 11                # ring: 8 per block + 3 slack (mixture lags a block)

# psum chunking: matmul N<=512 (one bank), ACT reads 4 banks
_CHUNKS_A = [(0, 2048), (2048, 2048), (4096, 2048)]
_CHUNKS_B = [(0, 2048), (2048, 2048), (4096, 2048), (6144, 212)]

F32 = mybir.dt.float32
BF16 = mybir.dt.bfloat16

_RUN_KWARGS = {}  # test harness may set trace/tmpdir here
_CACHE = {}


def _build():
    nc = bacc.Bacc("TRN2", target_bir_lowering=False, debug=False,
                   num_devices=N_CORES)
    xT = nc.dram_tensor("xT", [D, B], BF16, kind="ExternalInput").ap()
    pmT = nc.dram_tensor("pmT", [D, H * D], BF16, kind="ExternalInput").ap()
    mmT = nc.dram_tensor("mmT", [D, H], BF16, kind="ExternalInput").ap()
    embT = nc.dram_tensor("embT", [D, V_S], BF16, kind="ExternalInput").ap()
    out = nc.dram_tensor("out", [B, V_S], BF16, kind="ExternalOutput").ap()

    with tile.TileContext(nc) as tc:
        _body(tc, xT, pmT, mmT, embT, out)
        tc._pool_ctx.close()

    nc.compile()
    nc.m = get_hw_module(nc.m)
    return nc


def _body(tc, xT, pmT, mmT, embT, out):
    nc = tc.nc
    Exp = mybir.ActivationFunctionType.Exp
    Tanh = mybir.ActivationFunctionType.Tanh
    add = mybir.AluOpType.add
    mult = mybir.AluOpType.mult

    import contextlib
    ctx = contextlib.ExitStack()
    tc._pool_ctx = ctx
    singles = ctx.enter_context(tc.tile_pool(name="singles", bufs=1))
    work = ctx.enter_context(tc.tile_pool(name="work", bufs=3))
    lwp = ctx.enter_context(tc.tile_pool(name="lwp", bufs=2))
    ering = ctx.enter_context(tc.tile_pool(name="ering", bufs=E_SLOTS))
    psum = ctx.enter_context(tc.tile_pool(name="psum", bufs=2, space="PSUM"))
    dram = ctx.enter_context(tc.tile_pool(name="dram", bufs=4, space="DRAM"))

    # ---- resident SBUF inputs (xT/pmT borrow e-ring slots: prologue-only)
    sb_xT, sb_pmT, sb_mmT, sb_embT = [], [], [], []
    for k in range(KT):
        t = ering.tile([128, HALF_B], BF16, tag="e", name=f"xT{k}")
        nc.sync.dma_start(out=t[:, :B], in_=xT[k * 128:(k + 1) * 128, :])
        sb_xT.append(t[:, :B])
        t = ering.tile([128, HALF_B], BF16, tag="e", name=f"pmT{k}")
        nc.sync.dma_start(out=t[:, :H * D], in_=pmT[k * 128:(k + 1) * 128, :])
        sb_pmT.append(t[:, :H * D])
        t = work.tile([128, H], BF16, tag=f"mmT{k}", name=f"mmT{k}")
        nc.sync.dma_start(out=t, in_=mmT[k * 128:(k + 1) * 128, :])
        sb_mmT.append(t)
        t = singles.tile([128, V_S], BF16, tag=f"embT{k}", name=f"embT{k}")
        nc.sync.dma_start(out=t, in_=embT[k * 128:(k + 1) * 128, :])
        sb_embT.append(t)

    # ---- projT[h][kd] = tanh(proj_mat_h @ x.T), spilled to DRAM ----
    projT_dram = [[dram.tile([128, B], BF16, tag=f"pjd{h}_{kd}", bufs=1,
                             name=f"pjd{h}_{kd}")
                   for kd in range(KT)] for h in range(H)]
    for h in range(H):
        for kd in range(KT):
            for bs in range(B // 512):
                ps = psum.tile([128, 2048], F32, tag="ps", name="ps")
                for kc in range(KT):
                    nc.tensor.matmul(
                        ps[:, :512],
                        sb_pmT[kc][:, h * D + kd * 128: h * D + (kd + 1) * 128],
                        sb_xT[kc][:, bs * 512:(bs + 1) * 512],
                        start=(kc == 0), stop=(kc == KT - 1),
                    )
                stg = work.tile([128, 512], BF16, tag="stg", name="stg")
                nc.scalar.activation(out=stg, in_=ps[:, :512], func=Tanh)
                nc.sync.dma_start(
                    out=projT_dram[h][kd][:, bs * 512:(bs + 1) * 512],
                    in_=stg)

    # ---- pi[b, h] = softmax_h(x @ mix_mat.T) per b-block ----
    sb_pi = []
    for i in range(N_BBLK):
        ps = psum.tile([128, 2048], F32, tag="ps", name="ps")
        for kc in range(KT):
            nc.tensor.matmul(
                ps[:, :H],
                sb_xT[kc][:, i * 128:(i + 1) * 128],
                sb_mmT[kc],
                start=(kc == 0), stop=(kc == KT - 1),
            )
        m = work.tile([128, 1], F32, tag="pim", name="pim")
        nc.vector.tensor_reduce(out=m, in_=ps[:, :H],
                                axis=mybir.AxisListType.X,
                                op=mybir.AluOpType.max)
        negm = work.tile([128, 1], F32, tag="pinegm", name="pinegm")
        nc.vector.tensor_scalar_mul(negm, m, -1.0)
        e = work.tile([128, H], F32, tag="pie", name="pie")
        nc.scalar.activation(out=e, in_=ps[:, :H], func=Exp, bias=negm)
        s = work.tile([128, 1], F32, tag="pis", name="pis")
        nc.vector.tensor_reduce(out=s, in_=e, axis=mybir.AxisListType.X,
                                op=add)
        rs = work.tile([128, 1], F32, tag="pirs", name="pirs")
        nc.vector.reciprocal(rs, s)
        pi = singles.tile([128, H], F32, tag=f"pi{i}", name=f"pi{i}")
        nc.vector.tensor_scalar_mul(pi, e, rs)
        sb_pi.append(pi)

    # ---- main loop over b-blocks ----
    def load_weights(i):
        lw = {}
        for h in range(H):
            for kc in range(KT):
                t = lwp.tile([128, 128], BF16, tag=f"lw{h}_{kc}",
                             name=f"lw{h}_{kc}")
                nc.sync.dma_start(
                    out=t, in_=projT_dram[h][kc][:, i * 128:(i + 1) * 128])
                lw[(h, kc)] = t
        return lw

    def do_mixture(pend):
        """pi-weighted mixture for a completed block; lags production."""
        i, ehalves, s_glob = pend
        rS = work.tile([128, H], F32, tag="rS", name="rS")
        nc.vector.reciprocal(rS, s_glob)
        w = work.tile([128, H], F32, tag="w", name="w")
        nc.vector.tensor_mul(w, sb_pi[i], rS)
        for half, (v0, vw) in enumerate([(0, HALF_A), (HALF_A, HALF_B)]):
            # accumulate into head 3's tile (allocated last -> freed last,
            # which matches the ring's FIFO reuse order)
            acc = ehalves[(3, half)][:, :vw]
            nc.vector.tensor_scalar_mul(acc, acc, w[:, 3:4])
            for h in range(2, -1, -1):
                nc.vector.scalar_tensor_tensor(
                    out=acc, in0=ehalves[(h, half)][:, :vw],
                    scalar=w[:, h:h + 1], in1=acc, op0=mult, op1=add)
            nc.sync.dma_start(
                out=out[i * 128:(i + 1) * 128, v0:v0 + vw], in_=acc)

    lw_cur = load_weights(0)
    pend = None
    for i in range(N_BBLK):
        # issue the lagged mixture FIRST: the DVE queue is strict FIFO, and
        # this block's production reuses ring slots freed by these reads
        if pend is not None:
            do_mixture(pend)
            pend = None
        s_blk = work.tile([128, H], F32, tag="sblk", name="sblk")
        lw_next = None
        ehalves = {}
        for h in range(H):
            sparts = work.tile([128, 7], F32, tag=f"sp{h}", name=f"sp{h}")
            for half, (v0h, chunks) in enumerate(
                    [(0, _CHUNKS_A), (HALF_A, _CHUNKS_B)]):
                ehalf = ering.tile([128, HALF_B], BF16, tag="e",
                                   name=f"e{h}_{half}")
                ehalves[(h, half)] = ehalf
                for ci, (c0, cw) in enumerate(chunks):
                    v0 = v0h + c0
                    ps = psum.tile([128, 2048], F32, tag="ps", name="ps")
                    for kc in range(KT):
                        for ns in range((cw + 511) // 512):
                            n0 = ns * 512
                            nw = min(512, cw - n0)
                            nc.tensor.matmul(
                                ps[:, n0:n0 + nw],
                                lw_cur[(h, kc)],
                                sb_embT[kc][:, v0 + n0:v0 + n0 + nw],
                                start=(kc == 0), stop=(kc == KT - 1),
                            )
                    if ci < 3:
                        nc.scalar.activation(
                            out=ehalf[:, c0:c0 + cw], in_=ps[:, :cw],
                            func=Exp,
                            accum_out=sparts[:, half * 3 + ci:
                                             half * 3 + ci + 1],
                        )
                    else:
                        # tail chunk: skip ScalarE's accum register read;
                        # the 212-wide row-sum goes to DVE (has slack)
                        nc.scalar.activation(
                            out=ehalf[:, c0:c0 + cw], in_=ps[:, :cw],
                            func=Exp)
                        nc.vector.tensor_reduce(
                            out=sparts[:, 6:7],
                            in_=ehalf[:, c0:c0 + cw],
                            axis=mybir.AxisListType.X, op=add)
            nc.vector.tensor_reduce(
                out=s_blk[:, h:h + 1], in_=sparts,
                axis=mybir.AxisListType.X, op=add)
            if h == 0 and i + 1 < N_BBLK:
                # prefetch next block's weight slices during head 1
                lw_next = load_weights(i + 1)

        # one [128, H] AllReduce of all heads' denominators per block
        cc_in = dram.tile([128, H], F32, tag="ccin", name="ccin")
        cc_out = dram.tile([128, H], F32, tag="ccout", name="ccout")
        nc.gpsimd.dma_start(out=cc_in[:], in_=s_blk)
        nc.gpsimd.collective_compute(
            "AllReduce", add,
            replica_groups=[list(range(N_CORES))],
            ins=[cc_in.opt()], outs=[cc_out.opt()],
        )
        s_glob = work.tile([128, H], F32, tag="sglob", name="sglob")
        # gpsimd queue, NOT sync: the sync FIFO carries the big output
        # DMAs whose sem-waits would head-of-line-block this read
        nc.gpsimd.dma_start(out=s_glob, in_=cc_out[:])

        pend = (i, ehalves, s_glob)
        if lw_next is not None:
            lw_cur = lw_next
    do_mixture(pend)


def _get_nc():
    if "nc" not in _CACHE:
        _CACHE["nc"] = _build()
    return _CACHE["nc"]


def kernel(x, proj_mat, mix_mat, emb):
    nc = _get_nc()
    bf = ml_dtypes.bfloat16
    xT = np.ascontiguousarray(x.astype(bf).T)
    pmT = np.ascontiguousarray(proj_mat.astype(bf).T)
    mmT = np.ascontiguousarray(mix_mat.astype(bf).T)
    emb_bf = emb.astype(bf)
    in_maps = []
    for c in range(N_CORES):
        embT = np.ascontiguousarray(emb_bf[c * V_S:(c + 1) * V_S].T)
        in_maps.append({"xT": xT, "pmT": pmT, "mmT": mmT, "embT": embT})
    res = run_bass_kernel_spmd(nc, in_maps, list(range(N_CORES)),
                               **_RUN_KWARGS)
    _CACHE["last_result"] = res
    return np.concatenate(
        [res.results[c]["out"].astype(np.float32) for c in range(N_CORES)],
        axis=1)


# revision 6
# speedup vs baseline: 1.2854x; 1.0578x over previous
"""Mixture-of-Softmaxes kernel for 8 Trainium2 NeuronCores.

Strategy: tensor-parallel over the vocab dimension (V=100000 -> 12500/core).
Each core computes all B rows for its vocab shard: per-head logits via fp8
DoubleRow matmuls (K=256 in one pass, 2 fp8 MACs/cell/cycle -- the PE is
power-throttled to ~1.95GHz when all engines run, so halving its cycles
matters), exp via ScalarE with row-sum side-accumulation, ONE per-block
[128,4] AllReduce of all four heads' softmax denominators (8 collectives
total -- the CC engine's ~20us/op service time paced the per-head version),
then a pi-weighted mixture on VectorE that lags production by one block.

fp8 packing: emb is scaled x64 on the host (keeps values out of fp8e4m3's
subnormal range; the exp un-scales via its free scale operand) and packed
[128, 2, 12512] with the two K-subtiles interleaved in the free dim
(DoubleRow 3D-AP layout, 16B-aligned substride). proj stays within fp8
normal range naturally (tanh output), quantized directly by the tanh
activation's output cast.

Mixture: in-place 4x tensor_scalar per head then a 2x tensor_tensor chain
into the last head's e-tile (freed last by ring order), so no extra
accumulator SBUF; the e-ring runs 12 slots deep.

Host-side prep: inputs are transposed (contraction dim -> SBUF partitions)
and cast before DMA, so the kernel needs no on-chip transposes.
"""

import numpy as np
import ml_dtypes

import concourse.bass as bass
import concourse.mybir as mybir
import concourse.tile as tile
from concourse import bacc
from concourse.bass_utils import run_bass_kernel_spmd
from concourse.bass_interp import get_hw_module

B, H, D, V = 1024, 4, 256, 100000
N_CORES = 8
V_S = V // N_CORES          # 12500 vocab entries per core
KT = D // 128               # 2 contraction k-tiles
BBLK = 128                  # b rows per block (= SBUF partitions)
N_BBLK = B // BBLK          # 8 blocks
HALF_A = 6144               # 3 psum chunks, no tail
HALF_B = V_S - HALF_A       # 6356 = 3 psum chunks + 212 tail
E_SLOTS = 12                # ring: 8 per block + 4 slack (mixture lags a block)
PADV = 12512                # V_S padded so the DoubleRow substride is 16B-aligned
EMB_SCALE = 64.0            # host-side emb scale; exp applies 1/EMB_SCALE

# psum chunking: matmul N<=512 (one bank), ACT reads 4 banks
_CHUNKS_A = [(0, 2048), (2048, 2048), (4096, 2048)]
_CHUNKS_B = [(0, 2048), (2048, 2048), (4096, 2048), (6144, 212)]

F32 = mybir.dt.float32
BF16 = mybir.dt.bfloat16
FP8 = mybir.dt.float8e4

_RUN_KWARGS = {}  # test harness may set trace/tmpdir here
_CACHE = {}


def _build():
    nc = bacc.Bacc("TRN2", target_bir_lowering=False, debug=False,
                   num_devices=N_CORES)
    xT = nc.dram_tensor("xT", [D, B], BF16, kind="ExternalInput").ap()
    pmT = nc.dram_tensor("pmT", [D, H * D], BF16, kind="ExternalInput").ap()
    mmT = nc.dram_tensor("mmT", [D, H], BF16, kind="ExternalInput").ap()
    embT = nc.dram_tensor("embT", [128, KT, PADV], FP8,
                          kind="ExternalInput").ap()
    out = nc.dram_tensor("out", [B, V_S], BF16, kind="ExternalOutput").ap()

    with tile.TileContext(nc) as tc:
        _body(tc, xT, pmT, mmT, embT, out)
        tc._pool_ctx.close()

    nc.compile()
    nc.m = get_hw_module(nc.m)
    return nc


def _body(tc, xT, pmT, mmT, embT, out):
    nc = tc.nc
    Exp = mybir.ActivationFunctionType.Exp
    Tanh = mybir.ActivationFunctionType.Tanh
    add = mybir.AluOpType.add
    DR = mybir.MatmulPerfMode.DoubleRow

    import contextlib
    ctx = contextlib.ExitStack()
    tc._pool_ctx = ctx
    singles = ctx.enter_context(tc.tile_pool(name="singles", bufs=1))
    work = ctx.enter_context(tc.tile_pool(name="work", bufs=3))
    lwp = ctx.enter_context(tc.tile_pool(name="lwp", bufs=2))
    ering = ctx.enter_context(tc.tile_pool(name="ering", bufs=E_SLOTS))
    psum = ctx.enter_context(tc.tile_pool(name="psum", bufs=2, space="PSUM"))
    dram = ctx.enter_context(tc.tile_pool(name="dram", bufs=4, space="DRAM"))

    # ---- resident SBUF inputs (xT/pmT borrow e-ring slots: prologue-only)
    sb_xT, sb_pmT, sb_mmT = [], [], []
    for k in range(KT):
        t = ering.tile([128, HALF_B], BF16, tag="e", name=f"xT{k}")
        nc.sync.dma_start(out=t[:, :B], in_=xT[k * 128:(k + 1) * 128, :])
        sb_xT.append(t[:, :B])
        t = ering.tile([128, HALF_B], BF16, tag="e", name=f"pmT{k}")
        nc.sync.dma_start(out=t[:, :H * D], in_=pmT[k * 128:(k + 1) * 128, :])
        sb_pmT.append(t[:, :H * D])
        t = work.tile([128, H], BF16, tag=f"mmT{k}", name=f"mmT{k}")
        nc.sync.dma_start(out=t, in_=mmT[k * 128:(k + 1) * 128, :])
        sb_mmT.append(t)
    sb_emb = singles.tile([128, KT, PADV], FP8, tag="embT", name="embT")
    nc.sync.dma_start(out=sb_emb, in_=embT)

    # ---- projT[h][kd] = tanh(proj_mat_h @ x.T) -> fp8, spilled to DRAM ----
    # bs outer so block 0's weight slices are spilled first
    projT_dram = [[dram.tile([128, B], FP8, tag=f"pjd{h}_{kd}", bufs=1,
                             name=f"pjd{h}_{kd}")
                   for kd in range(KT)] for h in range(H)]
    for bs in range(B // 512):
        for h in range(H):
            for kd in range(KT):
                ps = psum.tile([128, 2048], F32, tag="ps", name="ps")
                for kc in range(KT):
                    nc.tensor.matmul(
                        ps[:, :512],
                        sb_pmT[kc][:, h * D + kd * 128: h * D + (kd + 1) * 128],
                        sb_xT[kc][:, bs * 512:(bs + 1) * 512],
                        start=(kc == 0), stop=(kc == KT - 1),
                    )
                stg = work.tile([128, 512], FP8, tag="stg", name="stg")
                nc.scalar.activation(out=stg, in_=ps[:, :512], func=Tanh)
                nc.sync.dma_start(
                    out=projT_dram[h][kd][:, bs * 512:(bs + 1) * 512],
                    in_=stg)

    # ---- pi[b, h] = softmax_h(x @ mix_mat.T) per b-block ----
    sb_pi = []
    for i in range(N_BBLK):
        ps = psum.tile([128, 2048], F32, tag="ps", name="ps")
        for kc in range(KT):
            nc.tensor.matmul(
                ps[:, :H],
                sb_xT[kc][:, i * 128:(i + 1) * 128],
                sb_mmT[kc],
                start=(kc == 0), stop=(kc == KT - 1),
            )
        m = work.tile([128, 1], F32, tag="pim", name="pim")
        nc.vector.tensor_reduce(out=m, in_=ps[:, :H],
                                axis=mybir.AxisListType.X,
                                op=mybir.AluOpType.max)
        negm = work.tile([128, 1], F32, tag="pinegm", name="pinegm")
        nc.vector.tensor_scalar_mul(negm, m, -1.0)
        e = work.tile([128, H], F32, tag="pie", name="pie")
        nc.scalar.activation(out=e, in_=ps[:, :H], func=Exp, bias=negm)
        s = work.tile([128, 1], F32, tag="pis", name="pis")
        nc.vector.tensor_reduce(out=s, in_=e, axis=mybir.AxisListType.X,
                                op=add)
        rs = work.tile([128, 1], F32, tag="pirs", name="pirs")
        nc.vector.reciprocal(rs, s)
        pi = singles.tile([128, H], F32, tag=f"pi{i}", name=f"pi{i}")
        nc.vector.tensor_scalar_mul(pi, e, rs)
        sb_pi.append(pi)

    # ---- main loop over b-blocks ----
    def load_weights(i):
        lw = {}
        for h in range(H):
            t = lwp.tile([128, KT, 128], FP8, tag=f"lw{h}", name=f"lw{h}")
            for kc in range(KT):
                nc.sync.dma_start(
                    out=t[:, kc, :],
                    in_=projT_dram[h][kc][:, i * 128:(i + 1) * 128])
            lw[h] = t
        return lw

    def do_mixture(pend):
        """pi-weighted mixture for a completed block; lags production."""
        i, ehalves, s_glob = pend
        rS = work.tile([128, H], F32, tag="rS", name="rS")
        nc.vector.reciprocal(rS, s_glob)
        w = work.tile([128, H], F32, tag="w", name="w")
        nc.vector.tensor_mul(w, sb_pi[i], rS)
        for half, (v0, vw) in enumerate([(0, HALF_A), (HALF_A, HALF_B)]):
            # scale each head in place (4x mode), then add-chain (2x mode)
            # into head 3's tile (allocated last -> freed last, matching
            # the ring's FIFO reuse order)
            eh = [ehalves[(h, half)][:, :vw] for h in range(H)]
            for h in range(H):
                nc.vector.tensor_scalar_mul(eh[h], eh[h], w[:, h:h + 1])
            for h in range(2, -1, -1):
                nc.vector.tensor_tensor(out=eh[3], in0=eh[3], in1=eh[h],
                                        op=add)
            nc.sync.dma_start(
                out=out[i * 128:(i + 1) * 128, v0:v0 + vw], in_=eh[3])

    lw_cur = load_weights(0)
    pend = None
    for i in range(N_BBLK):
        # issue the lagged mixture FIRST: the DVE queue is strict FIFO, and
        # this block's production reuses ring slots freed by these reads
        if pend is not None:
            do_mixture(pend)
            pend = None
        s_blk = work.tile([128, H], F32, tag="sblk", name="sblk")
        lw_next = None
        ehalves = {}
        for h in range(H):
            sparts = work.tile([128, 7], F32, tag=f"sp{h}", name=f"sp{h}")
            for half, (v0h, chunks) in enumerate(
                    [(0, _CHUNKS_A), (HALF_A, _CHUNKS_B)]):
                ehalf = ering.tile([128, HALF_B], BF16, tag="e",
                                   name=f"e{h}_{half}")
                ehalves[(h, half)] = ehalf
                for ci, (c0, cw) in enumerate(chunks):
                    v0 = v0h + c0
                    ps = psum.tile([128, 2048], F32, tag="ps", name="ps")
                    for ns in range((cw + 511) // 512):
                        n0 = ns * 512
                        nw = min(512, cw - n0)
                        nc.tensor.matmul(
                            ps[:, n0:n0 + nw],
                            lw_cur[h],
                            sb_emb[:, :, v0 + n0:v0 + n0 + nw],
                            start=True, stop=True, perf_mode=DR,
                        )
                    if ci < 3:
                        nc.scalar.activation(
                            out=ehalf[:, c0:c0 + cw], in_=ps[:, :cw],
                            func=Exp, scale=1.0 / EMB_SCALE,
                            accum_out=sparts[:, half * 3 + ci:
                                             half * 3 + ci + 1],
                        )
                    else:
                        # tail chunk: skip ScalarE's accum register read;
                        # the 212-wide row-sum goes to DVE (has slack)
                        nc.scalar.activation(
                            out=ehalf[:, c0:c0 + cw], in_=ps[:, :cw],
                            func=Exp, scale=1.0 / EMB_SCALE)
                        nc.vector.tensor_reduce(
                            out=sparts[:, 6:7],
                            in_=ehalf[:, c0:c0 + cw],
                            axis=mybir.AxisListType.X, op=add)
            nc.vector.tensor_reduce(
                out=s_blk[:, h:h + 1], in_=sparts,
                axis=mybir.AxisListType.X, op=add)
            if h == 0 and i + 1 < N_BBLK:
                # prefetch next block's weight slices during head 1
                lw_next = load_weights(i + 1)

        # one [128, H] AllReduce of all heads' denominators per block
        cc_in = dram.tile([128, H], F32, tag="ccin", name="ccin")
        cc_out = dram.tile([128, H], F32, tag="ccout", name="ccout")
        nc.gpsimd.dma_start(out=cc_in[:], in_=s_blk)
        nc.gpsimd.collective_compute(
            "AllReduce", add,
            replica_groups=[list(range(N_CORES))],
            ins=[cc_in.opt()], outs=[cc_out.opt()],
        )
        s_glob = work.tile([128, H], F32, tag="sglob", name="sglob")
        # gpsimd queue, NOT sync: the sync FIFO carries the big output
        # DMAs whose sem-waits would head-of-line-block this read
        nc.gpsimd.dma_start(out=s_glob, in_=cc_out[:])

        pend = (i, ehalves, s_glob)
        if lw_next is not None:
            lw_cur = lw_next
    do_mixture(pend)


def _get_nc():
    if "nc" not in _CACHE:
        _CACHE["nc"] = _build()
    return _CACHE["nc"]


def kernel(x, proj_mat, mix_mat, emb):
    nc = _get_nc()
    bf = ml_dtypes.bfloat16
    f8 = ml_dtypes.float8_e4m3fn
    xT = np.ascontiguousarray(x.astype(bf).T)
    pmT = np.ascontiguousarray(proj_mat.astype(bf).T)
    mmT = np.ascontiguousarray(mix_mat.astype(bf).T)
    emb_s = (emb.astype(np.float32) * EMB_SCALE).astype(f8)
    in_maps = []
    for c in range(N_CORES):
        sub = emb_s[c * V_S:(c + 1) * V_S]          # [V_S, D]
        embT = np.zeros((128, KT, PADV), dtype=f8)
        for j in range(KT):
            embT[:, j, :V_S] = sub[:, j * 128:(j + 1) * 128].T
        in_maps.append({"xT": xT, "pmT": pmT, "mmT": mmT, "embT": embT})
    res = run_bass_kernel_spmd(nc, in_maps, list(range(N_CORES)),
                               **_RUN_KWARGS)
    _CACHE["last_result"] = res
    return np.concatenate(
        [res.results[c]["out"].astype(np.float32) for c in range(N_CORES)],
        axis=1)


# revision 10
# speedup vs baseline: 1.2884x; 1.0023x over previous
"""Mixture-of-Softmaxes kernel for 8 Trainium2 NeuronCores.

Strategy: tensor-parallel over the vocab dimension (V=100000 -> 12500/core).
Each core computes all B rows for its vocab shard: per-head logits via fp8
DoubleRow matmuls (K=256 in one pass, 2 fp8 MACs/cell/cycle -- the PE is
power-throttled to ~1.95GHz when all engines run, so halving its cycles
matters), exp via ScalarE with row-sum side-accumulation, ONE per-block
[128,4] AllReduce of all four heads' softmax denominators (8 collectives
total -- the CC engine's ~20us/op service time paced the per-head version),
then a pi-weighted mixture on VectorE that lags production by one block.

fp8 packing: emb is scaled x64 on the host (keeps values out of fp8e4m3's
subnormal range; the exp un-scales via its free scale operand) and packed
[128, 2, 12512] with the two K-subtiles interleaved in the free dim
(DoubleRow 3D-AP layout, 16B-aligned substride). proj stays within fp8
normal range naturally (tanh output), quantized directly by the tanh
activation's output cast.

Mixture: in-place 4x tensor_scalar per head then a 2x tensor_tensor chain
into the last head's e-tile (freed last by ring order), so no extra
accumulator SBUF; the e-ring runs 12 slots deep.

Host-side prep: inputs are transposed (contraction dim -> SBUF partitions)
and cast before DMA, so the kernel needs no on-chip transposes.
"""

import numpy as np
import ml_dtypes

import concourse.bass as bass
import concourse.mybir as mybir
import concourse.tile as tile
from concourse import bacc
from concourse.bass_utils import run_bass_kernel_spmd
from concourse.bass_interp import get_hw_module

B, H, D, V = 1024, 4, 256, 100000
N_CORES = 8
V_S = V // N_CORES          # 12500 vocab entries per core
KT = D // 128               # 2 contraction k-tiles
BBLK = 128                  # b rows per block (= SBUF partitions)
N_BBLK = B // BBLK          # 8 blocks
HALF_A = 6144               # 3 psum chunks, no tail
HALF_B = V_S - HALF_A       # 6356 = 3 psum chunks + 212 tail
E_SLOTS = 12                # ring: 8 per block + 4 slack (mixture lags a block)
PADV = 12512                # V_S padded so the DoubleRow substride is 16B-aligned
EMB_SCALE = 64.0            # host-side emb scale; exp applies 1/EMB_SCALE

# psum chunking: matmul N<=512 (one bank), ACT reads 4 banks
_CHUNKS_A = [(0, 2048), (2048, 2048), (4096, 2048)]
_CHUNKS_B = [(0, 2048), (2048, 2048), (4096, 2048), (6144, 212)]

F32 = mybir.dt.float32
BF16 = mybir.dt.bfloat16
FP8 = mybir.dt.float8e4

_RUN_KWARGS = {}  # test harness may set trace/tmpdir here
_CACHE = {}


def _enable_ldw_opt():
    """Let walrus dedupe back-to-back identical LDWEIGHTS (26 matmuls per
    (block, head) share one stationary tile). Requires waits to stay off
    the ldweights instructions, so _build also no-ops bacc's
    move_matmul_waits_to_ldweights pass."""
    from concourse import bass_utils as _bu
    if getattr(_bu, "_ldwopt_patched", False):
        return
    _orig = _bu.run_command

    def _patched(cmd, *a, **kw):
        cmd = [c.replace("--enable-ldw-opt=false", "--enable-ldw-opt=true")
               if isinstance(c, str) else c for c in cmd]
        return _orig(cmd, *a, **kw)

    _bu.run_command = _patched
    _bu._ldwopt_patched = True


def _build():
    nc = bacc.Bacc("TRN2", target_bir_lowering=False, debug=False,
                   num_devices=N_CORES)
    xT = nc.dram_tensor("xT", [D, B], BF16, kind="ExternalInput").ap()
    pmT = nc.dram_tensor("pmT", [D, H * D], BF16, kind="ExternalInput").ap()
    mmT = nc.dram_tensor("mmT", [D, H], BF16, kind="ExternalInput").ap()
    embT = nc.dram_tensor("embT", [128, KT, PADV], FP8,
                          kind="ExternalInput").ap()
    out = nc.dram_tensor("out", [B, V_S], BF16, kind="ExternalOutput").ap()

    with tile.TileContext(nc) as tc:
        _body(tc, xT, pmT, mmT, embT, out)
        tc._pool_ctx.close()

    nc.compile()
    nc.m = get_hw_module(nc.m)
    return nc


def _body(tc, xT, pmT, mmT, embT, out):
    nc = tc.nc
    Exp = mybir.ActivationFunctionType.Exp
    Tanh = mybir.ActivationFunctionType.Tanh
    add = mybir.AluOpType.add
    DR = mybir.MatmulPerfMode.DoubleRow

    import contextlib
    ctx = contextlib.ExitStack()
    tc._pool_ctx = ctx
    singles = ctx.enter_context(tc.tile_pool(name="singles", bufs=1))
    work = ctx.enter_context(tc.tile_pool(name="work", bufs=3))
    lwp = ctx.enter_context(tc.tile_pool(name="lwp", bufs=2))
    ering = ctx.enter_context(tc.tile_pool(name="ering", bufs=E_SLOTS))
    psum = ctx.enter_context(tc.tile_pool(name="psum", bufs=2, space="PSUM"))
    dram = ctx.enter_context(tc.tile_pool(name="dram", bufs=4, space="DRAM"))

    # ---- resident SBUF inputs (xT/pmT borrow e-ring slots: prologue-only)
    sb_xT, sb_pmT, sb_mmT = [], [], []
    for k in range(KT):
        t = ering.tile([128, HALF_B], BF16, tag="e", name=f"xT{k}")
        nc.sync.dma_start(out=t[:, :B], in_=xT[k * 128:(k + 1) * 128, :])
        sb_xT.append(t[:, :B])
        t = ering.tile([128, HALF_B], BF16, tag="e", name=f"pmT{k}")
        nc.sync.dma_start(out=t[:, :H * D], in_=pmT[k * 128:(k + 1) * 128, :])
        sb_pmT.append(t[:, :H * D])
        t = work.tile([128, H], BF16, tag=f"mmT{k}", name=f"mmT{k}")
        nc.sync.dma_start(out=t, in_=mmT[k * 128:(k + 1) * 128, :])
        sb_mmT.append(t)
    sb_emb = singles.tile([128, KT, PADV], FP8, tag="embT", name="embT")
    nc.sync.dma_start(out=sb_emb, in_=embT)

    # ---- projT[h][kd] = tanh(proj_mat_h @ x.T) -> fp8, spilled to DRAM ----
    # bs outer so block 0's weight slices are spilled first
    projT_dram = [[dram.tile([128, B], FP8, tag=f"pjd{h}_{kd}", bufs=1,
                             name=f"pjd{h}_{kd}")
                   for kd in range(KT)] for h in range(H)]
    for bs in range(B // 512):
        for h in range(H):
            for kd in range(KT):
                ps = psum.tile([128, 2048], F32, tag="ps", name="ps")
                for kc in range(KT):
                    nc.tensor.matmul(
                        ps[:, :512],
                        sb_pmT[kc][:, h * D + kd * 128: h * D + (kd + 1) * 128],
                        sb_xT[kc][:, bs * 512:(bs + 1) * 512],
                        start=(kc == 0), stop=(kc == KT - 1),
                    )
                stg = work.tile([128, 512], FP8, tag="stg", name="stg")
                nc.scalar.activation(out=stg, in_=ps[:, :512], func=Tanh)
                nc.sync.dma_start(
                    out=projT_dram[h][kd][:, bs * 512:(bs + 1) * 512],
                    in_=stg)

    # ---- pi[b, h] = softmax_h(x @ mix_mat.T) per b-block ----
    sb_pi = []
    for i in range(N_BBLK):
        ps = psum.tile([128, 2048], F32, tag="ps", name="ps")
        for kc in range(KT):
            nc.tensor.matmul(
                ps[:, :H],
                sb_xT[kc][:, i * 128:(i + 1) * 128],
                sb_mmT[kc],
                start=(kc == 0), stop=(kc == KT - 1),
            )
        m = work.tile([128, 1], F32, tag="pim", name="pim")
        nc.vector.tensor_reduce(out=m, in_=ps[:, :H],
                                axis=mybir.AxisListType.X,
                                op=mybir.AluOpType.max)
        negm = work.tile([128, 1], F32, tag="pinegm", name="pinegm")
        nc.vector.tensor_scalar_mul(negm, m, -1.0)
        e = work.tile([128, H], F32, tag="pie", name="pie")
        nc.scalar.activation(out=e, in_=ps[:, :H], func=Exp, bias=negm)
        s = work.tile([128, 1], F32, tag="pis", name="pis")
        nc.vector.tensor_reduce(out=s, in_=e, axis=mybir.AxisListType.X,
                                op=add)
        rs = work.tile([128, 1], F32, tag="pirs", name="pirs")
        nc.vector.reciprocal(rs, s)
        pi = singles.tile([128, H], F32, tag=f"pi{i}", name=f"pi{i}")
        nc.vector.tensor_scalar_mul(pi, e, rs)
        sb_pi.append(pi)

    # ---- main loop over b-blocks ----
    def all_reduce(src, tag):
        n = src.shape[-1]
        cc_in = dram.tile([128, n], F32, tag=f"ccin{tag}", name=f"ccin{tag}")
        cc_out = dram.tile([128, n], F32, tag=f"ccout{tag}",
                           name=f"ccout{tag}")
        nc.gpsimd.dma_start(out=cc_in[:], in_=src)
        nc.gpsimd.collective_compute(
            "AllReduce", add,
            replica_groups=[list(range(N_CORES))],
            ins=[cc_in.opt()], outs=[cc_out.opt()],
        )
        dst = work.tile([128, n], F32, tag=f"sglob{tag}", name=f"sglob{tag}")
        # gpsimd queue, NOT sync: the sync FIFO carries the big output
        # DMAs whose sem-waits would head-of-line-block this read
        nc.gpsimd.dma_start(out=dst, in_=cc_out[:])
        return dst

    def load_weights(i):
        lw = {}
        for h in range(H):
            t = lwp.tile([128, KT, 128], FP8, tag=f"lw{h}", name=f"lw{h}")
            for kc in range(KT):
                nc.sync.dma_start(
                    out=t[:, kc, :],
                    in_=projT_dram[h][kc][:, i * 128:(i + 1) * 128])
            lw[h] = t
        return lw

    def do_mixture(pend):
        """pi-weighted mixture for a completed block; lags production."""
        i, ehalves, s_glob = pend
        rS = work.tile([128, H], F32, tag="rS", name="rS")
        nc.vector.reciprocal(rS, s_glob)
        w = work.tile([128, H], F32, tag="w", name="w")
        nc.vector.tensor_mul(w, sb_pi[i], rS)
        for half, (v0, vw) in enumerate([(0, HALF_A), (HALF_A, HALF_B)]):
            # scale each head in place (4x mode), then add-chain (2x mode)
            # into head 3's tile (allocated last -> freed last, matching
            # the ring's FIFO reuse order)
            eh = [ehalves[(h, half)][:, :vw] for h in range(H)]
            for h in range(H):
                nc.vector.tensor_scalar_mul(eh[h], eh[h], w[:, h:h + 1])
            for h in range(2, -1, -1):
                nc.vector.tensor_tensor(out=eh[3], in0=eh[3], in1=eh[h],
                                        op=add)
            nc.sync.dma_start(
                out=out[i * 128:(i + 1) * 128, v0:v0 + vw], in_=eh[3])

    lw_cur = load_weights(0)
    pend = None
    for i in range(N_BBLK):
        # issue the lagged mixture FIRST: the DVE queue is strict FIFO, and
        # this block's production reuses ring slots freed by these reads
        if pend is not None:
            do_mixture(pend)
            pend = None
        s_blk = work.tile([128, H], F32, tag="sblk", name="sblk")
        lw_next = None
        ehalves = {}
        for h in range(H):
            sparts = work.tile([128, 7], F32, tag=f"sp{h}", name=f"sp{h}")
            for half, (v0h, chunks) in enumerate(
                    [(0, _CHUNKS_A), (HALF_A, _CHUNKS_B)]):
                ehalf = ering.tile([128, HALF_B], BF16, tag="e",
                                   name=f"e{h}_{half}")
                ehalves[(h, half)] = ehalf
                for ci, (c0, cw) in enumerate(chunks):
                    v0 = v0h + c0
                    ps = psum.tile([128, 2048], F32, tag="ps", name="ps")
                    for ns in range((cw + 511) // 512):
                        n0 = ns * 512
                        nw = min(512, cw - n0)
                        nc.tensor.matmul(
                            ps[:, n0:n0 + nw],
                            lw_cur[h],
                            sb_emb[:, :, v0 + n0:v0 + n0 + nw],
                            start=True, stop=True, perf_mode=DR,
                        )
                    if ci < 3:
                        nc.scalar.activation(
                            out=ehalf[:, c0:c0 + cw], in_=ps[:, :cw],
                            func=Exp, scale=1.0 / EMB_SCALE,
                            accum_out=sparts[:, half * 3 + ci:
                                             half * 3 + ci + 1],
                        )
                    else:
                        # tail chunk: skip ScalarE's accum register read;
                        # the 212-wide row-sum goes to DVE (has slack)
                        nc.scalar.activation(
                            out=ehalf[:, c0:c0 + cw], in_=ps[:, :cw],
                            func=Exp, scale=1.0 / EMB_SCALE)
                        nc.vector.tensor_reduce(
                            out=sparts[:, 6:7],
                            in_=ehalf[:, c0:c0 + cw],
                            axis=mybir.AxisListType.X, op=add)
            nc.vector.tensor_reduce(
                out=s_blk[:, h:h + 1], in_=sparts,
                axis=mybir.AxisListType.X, op=add)
            if h == 0 and i + 1 < N_BBLK:
                # prefetch next block's weight slices during head 1
                lw_next = load_weights(i + 1)
            if i == N_BBLK - 1 and h == 1:
                # last block: AllReduce heads 0-1 early so their mixture
                # weights are ready the moment production ends
                s_early = all_reduce(s_blk[:, 0:2], "01")

        if i < N_BBLK - 1:
            # one [128, H] AllReduce of all heads' denominators per block
            s_glob = all_reduce(s_blk, "b")
            pend = (i, ehalves, s_glob)
        else:
            s_late = all_reduce(s_blk[:, 2:4], "23")
        if lw_next is not None:
            lw_cur = lw_next

    # drain: last block's mixture, accumulated into head 0's tiles (ring
    # order no longer matters) so heads 0-1 start before AllReduce "23" lands
    def half_slices(h):
        return [ehalves[(h, 0)][:, :HALF_A], ehalves[(h, 1)][:, :HALF_B]]

    wts = []
    for tag, s_g, hs in [("01", s_early, (0, 1)), ("23", s_late, (2, 3))]:
        rS = work.tile([128, 2], F32, tag=f"rS{tag}", name=f"rS{tag}")
        nc.vector.reciprocal(rS, s_g)
        w = work.tile([128, 2], F32, tag=f"wl{tag}", name=f"wl{tag}")
        nc.vector.tensor_mul(w, sb_pi[N_BBLK - 1][:, hs[0]:hs[1] + 1], rS)
        for k, h in enumerate(hs):
            for eh in half_slices(h):
                nc.vector.tensor_scalar_mul(eh, eh, w[:, k:k + 1])
        a, b = half_slices(hs[0]), half_slices(hs[1])
        for half in range(2):
            nc.vector.tensor_tensor(out=a[half], in0=a[half], in1=b[half],
                                    op=add)
        wts.append(a)
    for half, (v0, vw) in enumerate([(0, HALF_A), (HALF_A, HALF_B)]):
        nc.vector.tensor_tensor(out=wts[0][half], in0=wts[0][half],
                                in1=wts[1][half], op=add)
        nc.sync.dma_start(
            out=out[(N_BBLK - 1) * 128:N_BBLK * 128, v0:v0 + vw],
            in_=wts[0][half])


def _get_nc():
    if "nc" not in _CACHE:
        _CACHE["nc"] = _build()
    return _CACHE["nc"]


def kernel(x, proj_mat, mix_mat, emb):
    nc = _get_nc()
    bf = ml_dtypes.bfloat16
    f8 = ml_dtypes.float8_e4m3fn
    xT = np.ascontiguousarray(x.astype(bf).T)
    pmT = np.ascontiguousarray(proj_mat.astype(bf).T)
    mmT = np.ascontiguousarray(mix_mat.astype(bf).T)
    emb_s = (emb.astype(np.float32) * EMB_SCALE).astype(f8)
    in_maps = []
    for c in range(N_CORES):
        sub = emb_s[c * V_S:(c + 1) * V_S]          # [V_S, D]
        embT = np.zeros((128, KT, PADV), dtype=f8)
        for j in range(KT):
            embT[:, j, :V_S] = sub[:, j * 128:(j + 1) * 128].T
        in_maps.append({"xT": xT, "pmT": pmT, "mmT": mmT, "embT": embT})
    res = run_bass_kernel_spmd(nc, in_maps, list(range(N_CORES)),
                               **_RUN_KWARGS)
    _CACHE["last_result"] = res
    return np.concatenate(
        [res.results[c]["out"].astype(np.float32) for c in range(N_CORES)],
        axis=1)


# revision 17
# speedup vs baseline: 1.4896x; 1.1562x over previous
"""Mixture-of-Softmaxes kernel for 8 Trainium2 NeuronCores.

Strategy: tensor-parallel over the vocab dimension (V=100000 -> 12500/core).
Each core computes all B rows for its vocab shard: per-head logits via fp8
DoubleRow matmuls (K=256 in one pass, 2 fp8 MACs/cell/cycle -- the PE is
power-throttled to ~1.95GHz when all engines run, so halving its cycles
matters), exp via ScalarE with row-sum side-accumulation, ONE per-block
[128,4] AllReduce of all four heads' softmax denominators (8 collectives
total -- the CC engine's ~20us/op service time paced the per-head version),
then a pi-weighted mixture on VectorE that lags production by one block.

fp8 packing: emb is scaled x64 on the host (keeps values out of fp8e4m3's
subnormal range; the exp un-scales via its free scale operand) and packed
[128, 2, 12512] with the two K-subtiles interleaved in the free dim
(DoubleRow 3D-AP layout, 16B-aligned substride). proj stays within fp8
normal range naturally (tanh output), quantized directly by the tanh
activation's output cast.

Mixture: in-place 4x tensor_scalar per head then a 2x tensor_tensor chain
into the last head's e-tile (freed last by ring order), so no extra
accumulator SBUF; the e-ring runs 12 slots deep.

Host-side prep: inputs are transposed (contraction dim -> SBUF partitions)
and cast before DMA, so the kernel needs no on-chip transposes.
"""

import numpy as np
import ml_dtypes

import concourse.bass as bass
import concourse.mybir as mybir
import concourse.tile as tile
from concourse import bacc
from concourse.bass_utils import run_bass_kernel_spmd
from concourse.bass_interp import get_hw_module

B, H, D, V = 1024, 4, 256, 100000
N_CORES = 8
V_S = V // N_CORES          # 12500 vocab entries per core
KT = D // 128               # 2 contraction k-tiles
BBLK = 128                  # b rows per block (= SBUF partitions)
N_BBLK = B // BBLK          # 8 blocks
HALF_A = 6144               # 3 psum chunks, no tail
HALF_B = V_S - HALF_A       # 6356 = 3 psum chunks + 212 tail
E_SLOTS = 13                # ring: 8 per block + 5 slack (mixture lags a block)
PADV = 12512                # V_S padded so the DoubleRow substride is 16B-aligned
EMB_SCALE = 64.0            # host-side emb scale; exp applies 1/EMB_SCALE

# psum chunking: matmul N<=512 (one bank), ACT reads 4 banks
_CHUNKS_A = [(0, 2048), (2048, 2048), (4096, 2048)]
_CHUNKS_B = [(0, 2048), (2048, 2048), (4096, 2048), (6144, 212)]

F32 = mybir.dt.float32
BF16 = mybir.dt.bfloat16
FP8 = mybir.dt.float8e4

_RUN_KWARGS = {}  # test harness may set trace/tmpdir here
_CACHE = {}


def _enable_ldw_opt():
    """Let walrus dedupe back-to-back identical LDWEIGHTS (26 matmuls per
    (block, head) share one stationary tile). Requires waits to stay off
    the ldweights instructions, so _build also no-ops bacc's
    move_matmul_waits_to_ldweights pass."""
    from concourse import bass_utils as _bu
    if getattr(_bu, "_ldwopt_patched", False):
        return
    _orig = _bu.run_command

    def _patched(cmd, *a, **kw):
        cmd = [c.replace("--enable-ldw-opt=false", "--enable-ldw-opt=true")
               if isinstance(c, str) else c for c in cmd]
        return _orig(cmd, *a, **kw)

    _bu.run_command = _patched
    _bu._ldwopt_patched = True


def _build():
    nc = bacc.Bacc("TRN2", target_bir_lowering=False, debug=False,
                   num_devices=N_CORES)
    xT = nc.dram_tensor("xT", [D, B], BF16, kind="ExternalInput").ap()
    pmT = nc.dram_tensor("pmT", [D, H * D], BF16, kind="ExternalInput").ap()
    mmT = nc.dram_tensor("mmT", [D, H], BF16, kind="ExternalInput").ap()
    embT = nc.dram_tensor("embT", [128, KT, PADV], FP8,
                          kind="ExternalInput").ap()
    out = nc.dram_tensor("out", [B, V_S], BF16, kind="ExternalOutput").ap()

    with tile.TileContext(nc) as tc:
        _body(tc, xT, pmT, mmT, embT, out)
        tc._pool_ctx.close()

    nc.compile()
    nc.m = get_hw_module(nc.m)
    return nc


def _body(tc, xT, pmT, mmT, embT, out):
    nc = tc.nc
    Exp = mybir.ActivationFunctionType.Exp
    Tanh = mybir.ActivationFunctionType.Tanh
    add = mybir.AluOpType.add
    DR = mybir.MatmulPerfMode.DoubleRow

    import contextlib
    ctx = contextlib.ExitStack()
    tc._pool_ctx = ctx
    singles = ctx.enter_context(tc.tile_pool(name="singles", bufs=1))
    work = ctx.enter_context(tc.tile_pool(name="work", bufs=3))
    lwp = ctx.enter_context(tc.tile_pool(name="lwp", bufs=2))
    ering = ctx.enter_context(tc.tile_pool(name="ering", bufs=E_SLOTS))
    psum = ctx.enter_context(tc.tile_pool(name="psum", bufs=2, space="PSUM"))
    dram = ctx.enter_context(tc.tile_pool(name="dram", bufs=4, space="DRAM"))

    # ---- resident SBUF inputs (xT/pmT borrow e-ring slots: prologue-only)
    sb_xT, sb_pmT, sb_mmT = [], [], []
    for k in range(KT):
        t = ering.tile([128, HALF_B], BF16, tag="e", name=f"xT{k}")
        nc.sync.dma_start(out=t[:, :B], in_=xT[k * 128:(k + 1) * 128, :])
        sb_xT.append(t[:, :B])
        t = ering.tile([128, HALF_B], BF16, tag="e", name=f"pmT{k}")
        nc.sync.dma_start(out=t[:, :H * D], in_=pmT[k * 128:(k + 1) * 128, :])
        sb_pmT.append(t[:, :H * D])
        t = work.tile([128, H], BF16, tag=f"mmT{k}", name=f"mmT{k}")
        nc.sync.dma_start(out=t, in_=mmT[k * 128:(k + 1) * 128, :])
        sb_mmT.append(t)
    sb_emb = singles.tile([128, KT, PADV], FP8, tag="embT", name="embT")
    nc.sync.dma_start(out=sb_emb, in_=embT)

    # ---- projT[h][kd] = tanh(proj_mat_h @ x.T) -> fp8, spilled to DRAM ----
    # bs outer so block 0's weight slices are spilled first
    projT_dram = [[dram.tile([128, B], FP8, tag=f"pjd{h}_{kd}", bufs=1,
                             name=f"pjd{h}_{kd}")
                   for kd in range(KT)] for h in range(H)]
    for bs in range(B // 512):
        for h in range(H):
            for kd in range(KT):
                ps = psum.tile([128, 2048], F32, tag="ps", name="ps")
                for kc in range(KT):
                    nc.tensor.matmul(
                        ps[:, :512],
                        sb_pmT[kc][:, h * D + kd * 128: h * D + (kd + 1) * 128],
                        sb_xT[kc][:, bs * 512:(bs + 1) * 512],
                        start=(kc == 0), stop=(kc == KT - 1),
                    )
                stg = work.tile([128, 512], FP8, tag="stg", name="stg")
                nc.scalar.activation(out=stg, in_=ps[:, :512], func=Tanh)
                nc.sync.dma_start(
                    out=projT_dram[h][kd][:, bs * 512:(bs + 1) * 512],
                    in_=stg)

    # ---- pi[b, h] = softmax_h(x @ mix_mat.T) per b-block ----
    sb_pi = []
    for i in range(N_BBLK):
        ps = psum.tile([128, 2048], F32, tag="ps", name="ps")
        for kc in range(KT):
            nc.tensor.matmul(
                ps[:, :H],
                sb_xT[kc][:, i * 128:(i + 1) * 128],
                sb_mmT[kc],
                start=(kc == 0), stop=(kc == KT - 1),
            )
        m = work.tile([128, 1], F32, tag="pim", name="pim")
        nc.vector.tensor_reduce(out=m, in_=ps[:, :H],
                                axis=mybir.AxisListType.X,
                                op=mybir.AluOpType.max)
        negm = work.tile([128, 1], F32, tag="pinegm", name="pinegm")
        nc.vector.tensor_scalar_mul(negm, m, -1.0)
        e = work.tile([128, H], F32, tag="pie", name="pie")
        nc.scalar.activation(out=e, in_=ps[:, :H], func=Exp, bias=negm)
        s = work.tile([128, 1], F32, tag="pis", name="pis")
        nc.vector.tensor_reduce(out=s, in_=e, axis=mybir.AxisListType.X,
                                op=add)
        rs = work.tile([128, 1], F32, tag="pirs", name="pirs")
        nc.vector.reciprocal(rs, s)
        pi = singles.tile([128, H], F32, tag=f"pi{i}", name=f"pi{i}")
        nc.vector.tensor_scalar_mul(pi, e, rs)
        sb_pi.append(pi)

    # ---- main loop over b-blocks ----
    def all_reduce(src, tag, nheads):
        """AllReduce the RAW per-chunk partials [128, nheads*7]; the 7->1
        per-head reduce happens on DVE after the collective (it is gated
        on the collective anyway, and keeping the DVE queue free of
        pre-collective work lets the collective launch at block end)."""
        n = nheads * 7
        cc_in = dram.tile([128, n], F32, tag=f"ccin{tag}", name=f"ccin{tag}")
        cc_out = dram.tile([128, n], F32, tag=f"ccout{tag}",
                           name=f"ccout{tag}")
        nc.gpsimd.dma_start(out=cc_in[:], in_=src)
        nc.gpsimd.collective_compute(
            "AllReduce", add,
            replica_groups=[list(range(N_CORES))],
            ins=[cc_in.opt()], outs=[cc_out.opt()],
        )
        dst = work.tile([128, nheads, 7], F32, tag=f"sglob{tag}",
                        name=f"sglob{tag}")
        # gpsimd queue, NOT sync: the sync FIFO carries the big output
        # DMAs whose sem-waits would head-of-line-block this read
        nc.gpsimd.dma_start(out=dst, in_=cc_out[:])
        return dst

    def load_weights(i):
        lw = {}
        for h in range(H):
            t = lwp.tile([128, KT, 128], FP8, tag=f"lw{h}", name=f"lw{h}")
            for kc in range(KT):
                nc.sync.dma_start(
                    out=t[:, kc, :],
                    in_=projT_dram[h][kc][:, i * 128:(i + 1) * 128])
            lw[h] = t
        return lw

    def do_mixture(pend):
        """pi-weighted mixture for a completed block; lags production."""
        i, ehalves, s_glob = pend
        s4 = work.tile([128, H], F32, tag="s4", name="s4")
        nc.vector.tensor_reduce(out=s4, in_=s_glob,
                                axis=mybir.AxisListType.X, op=add)
        rS = work.tile([128, H], F32, tag="rS", name="rS")
        nc.vector.reciprocal(rS, s4)
        w = work.tile([128, H], F32, tag="w", name="w")
        nc.vector.tensor_mul(w, sb_pi[i], rS)
        for half, (v0, vw) in enumerate([(0, HALF_A), (HALF_A, HALF_B)]):
            # scale each head in place (4x mode), then add-chain (2x mode)
            # into head 3's tile (allocated last -> freed last, matching
            # the ring's FIFO reuse order). Heads 0..2 are added in THAT
            # order: their tiles are the next ones the ring reuses, so
            # releasing them earliest unblocks the next block's ScalarE.
            eh = [ehalves[(h, half)][:, :vw] for h in range(H)]
            nc.vector.tensor_scalar_mul(eh[3], eh[3], w[:, 3:4])
            for h in range(3):
                nc.vector.tensor_scalar_mul(eh[h], eh[h], w[:, h:h + 1])
                nc.vector.tensor_tensor(out=eh[3], in0=eh[3], in1=eh[h],
                                        op=add)
            nc.sync.dma_start(
                out=out[i * 128:(i + 1) * 128, v0:v0 + vw], in_=eh[3])

    lw_cur = load_weights(0)
    pend = None
    for i in range(N_BBLK):
        # issue the lagged mixture FIRST: the DVE queue is strict FIFO, and
        # this block's production reuses ring slots freed by these reads
        if pend is not None:
            do_mixture(pend)
            pend = None
        sparts = work.tile([128, H * 7], F32, tag="spb", name="spb")
        lw_next = None
        ehalves = {}
        for h in range(H):
            for half, (v0h, chunks) in enumerate(
                    [(0, _CHUNKS_A), (HALF_A, _CHUNKS_B)]):
                ehalf = ering.tile([128, HALF_B], BF16, tag="e",
                                   name=f"e{h}_{half}")
                ehalves[(h, half)] = ehalf
                for ci, (c0, cw) in enumerate(chunks):
                    v0 = v0h + c0
                    ps = psum.tile([128, 2048], F32, tag="ps", name="ps")
                    for ns in range((cw + 511) // 512):
                        n0 = ns * 512
                        nw = min(512, cw - n0)
                        nc.tensor.matmul(
                            ps[:, n0:n0 + nw],
                            lw_cur[h],
                            sb_emb[:, :, v0 + n0:v0 + n0 + nw],
                            start=True, stop=True, perf_mode=DR,
                        )
                    cell = h * 7 + half * 3 + ci
                    nc.scalar.activation(
                        out=ehalf[:, c0:c0 + cw], in_=ps[:, :cw],
                        func=Exp, scale=1.0 / EMB_SCALE,
                        accum_out=sparts[:, cell:cell + 1],
                    )
            if h == 0 and i + 1 < N_BBLK:
                # prefetch next block's weight slices during head 1
                lw_next = load_weights(i + 1)
            if i == N_BBLK - 1 and h == 1:
                # last block: AllReduce heads 0-1 early so their mixture
                # weights are ready the moment production ends
                s_early = all_reduce(sparts[:, 0:14], "01", 2)

        if i < N_BBLK - 1:
            # one AllReduce of all heads' denominator partials per block
            s_glob = all_reduce(sparts, "b", H)
            pend = (i, ehalves, s_glob)
        else:
            s_late = all_reduce(sparts[:, 14:28], "23", 2)
        if lw_next is not None:
            lw_cur = lw_next

    # drain: last block's mixture, accumulated into head 0's tiles (ring
    # order no longer matters) so heads 0-1 start before AllReduce "23" lands
    def half_slices(h):
        return [ehalves[(h, 0)][:, :HALF_A], ehalves[(h, 1)][:, :HALF_B]]

    wts = []
    for tag, s_g, hs in [("01", s_early, (0, 1)), ("23", s_late, (2, 3))]:
        s2 = work.tile([128, 2], F32, tag=f"s2{tag}", name=f"s2{tag}")
        nc.vector.tensor_reduce(out=s2, in_=s_g,
                                axis=mybir.AxisListType.X, op=add)
        rS = work.tile([128, 2], F32, tag=f"rS{tag}", name=f"rS{tag}")
        nc.vector.reciprocal(rS, s2)
        w = work.tile([128, 2], F32, tag=f"wl{tag}", name=f"wl{tag}")
        nc.vector.tensor_mul(w, sb_pi[N_BBLK - 1][:, hs[0]:hs[1] + 1], rS)
        for k, h in enumerate(hs):
            for eh in half_slices(h):
                nc.vector.tensor_scalar_mul(eh, eh, w[:, k:k + 1])
        a, b = half_slices(hs[0]), half_slices(hs[1])
        for half in range(2):
            nc.vector.tensor_tensor(out=a[half], in0=a[half], in1=b[half],
                                    op=add)
        wts.append(a)
    for half, (v0, vw) in enumerate([(0, HALF_A), (HALF_A, HALF_B)]):
        nc.vector.tensor_tensor(out=wts[0][half], in0=wts[0][half],
                                in1=wts[1][half], op=add)
        nc.sync.dma_start(
            out=out[(N_BBLK - 1) * 128:N_BBLK * 128, v0:v0 + vw],
            in_=wts[0][half])


def _get_nc():
    if "nc" not in _CACHE:
        _CACHE["nc"] = _build()
    return _CACHE["nc"]


def kernel(x, proj_mat, mix_mat, emb):
    nc = _get_nc()
    bf = ml_dtypes.bfloat16
    f8 = ml_dtypes.float8_e4m3fn
    xT = np.ascontiguousarray(x.astype(bf).T)
    pmT = np.ascontiguousarray(proj_mat.astype(bf).T)
    mmT = np.ascontiguousarray(mix_mat.astype(bf).T)
    emb_s = (emb.astype(np.float32) * EMB_SCALE).astype(f8)
    in_maps = []
    for c in range(N_CORES):
        sub = emb_s[c * V_S:(c + 1) * V_S]          # [V_S, D]
        embT = np.zeros((128, KT, PADV), dtype=f8)
        for j in range(KT):
            embT[:, j, :V_S] = sub[:, j * 128:(j + 1) * 128].T
        in_maps.append({"xT": xT, "pmT": pmT, "mmT": mmT, "embT": embT})
    res = run_bass_kernel_spmd(nc, in_maps, list(range(N_CORES)),
                               **_RUN_KWARGS)
    _CACHE["last_result"] = res
    return np.concatenate(
        [res.results[c]["out"].astype(np.float32) for c in range(N_CORES)],
        axis=1)


# revision 19
# speedup vs baseline: 1.4924x; 1.0019x over previous
"""Mixture-of-Softmaxes kernel for 8 Trainium2 NeuronCores.

Strategy: tensor-parallel over the vocab dimension (V=100000 -> 12500/core).
Each core computes all B rows for its vocab shard: per-head logits via fp8
DoubleRow matmuls (K=256 in one pass, 2 fp8 MACs/cell/cycle -- the PE is
power-throttled to ~1.95GHz when all engines run, so halving its cycles
matters), exp via ScalarE with row-sum side-accumulation, ONE per-block
[128,4] AllReduce of all four heads' softmax denominators (8 collectives
total -- the CC engine's ~20us/op service time paced the per-head version),
then a pi-weighted mixture on VectorE that lags production by one block.

fp8 packing: emb is scaled x64 on the host (keeps values out of fp8e4m3's
subnormal range; the exp un-scales via its free scale operand) and packed
[128, 2, 12512] with the two K-subtiles interleaved in the free dim
(DoubleRow 3D-AP layout, 16B-aligned substride). proj stays within fp8
normal range naturally (tanh output), quantized directly by the tanh
activation's output cast.

Mixture: in-place 4x tensor_scalar per head then a 2x tensor_tensor chain
into the last head's e-tile (freed last by ring order), so no extra
accumulator SBUF; the e-ring runs 12 slots deep.

Host-side prep: inputs are transposed (contraction dim -> SBUF partitions)
and cast before DMA, so the kernel needs no on-chip transposes.
"""

import numpy as np
import ml_dtypes

import concourse.bass as bass
import concourse.mybir as mybir
import concourse.tile as tile
from concourse import bacc
from concourse.bass_utils import run_bass_kernel_spmd
from concourse.bass_interp import get_hw_module

B, H, D, V = 1024, 4, 256, 100000
N_CORES = 8
V_S = V // N_CORES          # 12500 vocab entries per core
KT = D // 128               # 2 contraction k-tiles
BBLK = 128                  # b rows per block (= SBUF partitions)
N_BBLK = B // BBLK          # 8 blocks
HALF_A = 6144               # 3 psum chunks, no tail
HALF_B = V_S - HALF_A       # 6356 = 3 psum chunks + 212 tail
E_SLOTS = 13                # ring: 8 per block + 5 slack (mixture lags a block)
PADV = 12512                # V_S padded so the DoubleRow substride is 16B-aligned
EMB_SCALE = 64.0            # host-side emb scale; exp applies 1/EMB_SCALE

# psum chunking: matmul N<=512 (one bank), ACT reads 4 banks
_CHUNKS_A = [(0, 2048), (2048, 2048), (4096, 2048)]
_CHUNKS_B = [(0, 2048), (2048, 2048), (4096, 2048), (6144, 212)]

F32 = mybir.dt.float32
BF16 = mybir.dt.bfloat16
FP8 = mybir.dt.float8e4

_RUN_KWARGS = {}  # test harness may set trace/tmpdir here
_CACHE = {}


def _enable_ldw_opt():
    """Let walrus dedupe back-to-back identical LDWEIGHTS (26 matmuls per
    (block, head) share one stationary tile). Requires waits to stay off
    the ldweights instructions, so _build also no-ops bacc's
    move_matmul_waits_to_ldweights pass."""
    from concourse import bass_utils as _bu
    if getattr(_bu, "_ldwopt_patched", False):
        return
    _orig = _bu.run_command

    def _patched(cmd, *a, **kw):
        cmd = [c.replace("--enable-ldw-opt=false", "--enable-ldw-opt=true")
               if isinstance(c, str) else c for c in cmd]
        return _orig(cmd, *a, **kw)

    _bu.run_command = _patched
    _bu._ldwopt_patched = True


def _build():
    nc = bacc.Bacc("TRN2", target_bir_lowering=False, debug=False,
                   num_devices=N_CORES)
    xT = nc.dram_tensor("xT", [D, B], BF16, kind="ExternalInput").ap()
    pmT = nc.dram_tensor("pmT", [D, H * D], BF16, kind="ExternalInput").ap()
    mmT = nc.dram_tensor("mmT", [D, H], BF16, kind="ExternalInput").ap()
    embT = nc.dram_tensor("embT", [128, KT, PADV], FP8,
                          kind="ExternalInput").ap()
    out = nc.dram_tensor("out", [B, V_S], BF16, kind="ExternalOutput").ap()

    with tile.TileContext(nc) as tc:
        _body(tc, xT, pmT, mmT, embT, out)
        tc._pool_ctx.close()

    nc.compile()
    nc.m = get_hw_module(nc.m)
    return nc


def _body(tc, xT, pmT, mmT, embT, out):
    nc = tc.nc
    Exp = mybir.ActivationFunctionType.Exp
    Tanh = mybir.ActivationFunctionType.Tanh
    add = mybir.AluOpType.add
    DR = mybir.MatmulPerfMode.DoubleRow

    import contextlib
    ctx = contextlib.ExitStack()
    tc._pool_ctx = ctx
    singles = ctx.enter_context(tc.tile_pool(name="singles", bufs=1))
    work = ctx.enter_context(tc.tile_pool(name="work", bufs=3))
    lwp = ctx.enter_context(tc.tile_pool(name="lwp", bufs=2))
    ering = ctx.enter_context(tc.tile_pool(name="ering", bufs=E_SLOTS))
    psum = ctx.enter_context(tc.tile_pool(name="psum", bufs=2, space="PSUM"))
    dram = ctx.enter_context(tc.tile_pool(name="dram", bufs=4, space="DRAM"))

    # ---- resident SBUF inputs (xT/pmT borrow e-ring slots: prologue-only)
    sb_xT, sb_pmT, sb_mmT = [], [], []
    for k in range(KT):
        t = ering.tile([128, HALF_B], BF16, tag="e", name=f"xT{k}")
        nc.sync.dma_start(out=t[:, :B], in_=xT[k * 128:(k + 1) * 128, :])
        sb_xT.append(t[:, :B])
        t = ering.tile([128, HALF_B], BF16, tag="e", name=f"pmT{k}")
        nc.sync.dma_start(out=t[:, :H * D], in_=pmT[k * 128:(k + 1) * 128, :])
        sb_pmT.append(t[:, :H * D])
        t = work.tile([128, H], BF16, tag=f"mmT{k}", name=f"mmT{k}")
        nc.sync.dma_start(out=t, in_=mmT[k * 128:(k + 1) * 128, :])
        sb_mmT.append(t)
    sb_emb = singles.tile([128, KT, PADV], FP8, tag="embT", name="embT")
    nc.sync.dma_start(out=sb_emb, in_=embT)

    # ---- projT[h][kd] = tanh(proj_mat_h @ x.T) -> fp8, spilled to DRAM ----
    # bs outer so block 0's weight slices are spilled first
    projT_dram = [[dram.tile([128, B], FP8, tag=f"pjd{h}_{kd}", bufs=1,
                             name=f"pjd{h}_{kd}")
                   for kd in range(KT)] for h in range(H)]
    for bs in range(B // 512):
        for h in range(H):
            for kd in range(KT):
                ps = psum.tile([128, 2048], F32, tag="ps", name="ps")
                for kc in range(KT):
                    nc.tensor.matmul(
                        ps[:, :512],
                        sb_pmT[kc][:, h * D + kd * 128: h * D + (kd + 1) * 128],
                        sb_xT[kc][:, bs * 512:(bs + 1) * 512],
                        start=(kc == 0), stop=(kc == KT - 1),
                    )
                stg = work.tile([128, 512], FP8, tag="stg", name="stg")
                nc.scalar.activation(out=stg, in_=ps[:, :512], func=Tanh)
                nc.sync.dma_start(
                    out=projT_dram[h][kd][:, bs * 512:(bs + 1) * 512],
                    in_=stg)

    # ---- pi[b, h] = softmax_h(x @ mix_mat.T) per b-block ----
    sb_pi = []
    for i in range(N_BBLK):
        ps = psum.tile([128, 2048], F32, tag="ps", name="ps")
        for kc in range(KT):
            nc.tensor.matmul(
                ps[:, :H],
                sb_xT[kc][:, i * 128:(i + 1) * 128],
                sb_mmT[kc],
                start=(kc == 0), stop=(kc == KT - 1),
            )
        m = work.tile([128, 1], F32, tag="pim", name="pim")
        nc.vector.tensor_reduce(out=m, in_=ps[:, :H],
                                axis=mybir.AxisListType.X,
                                op=mybir.AluOpType.max)
        negm = work.tile([128, 1], F32, tag="pinegm", name="pinegm")
        nc.vector.tensor_scalar_mul(negm, m, -1.0)
        e = work.tile([128, H], F32, tag="pie", name="pie")
        nc.scalar.activation(out=e, in_=ps[:, :H], func=Exp, bias=negm)
        s = work.tile([128, 1], F32, tag="pis", name="pis")
        nc.vector.tensor_reduce(out=s, in_=e, axis=mybir.AxisListType.X,
                                op=add)
        rs = work.tile([128, 1], F32, tag="pirs", name="pirs")
        nc.vector.reciprocal(rs, s)
        pi = singles.tile([128, H], F32, tag=f"pi{i}", name=f"pi{i}")
        nc.vector.tensor_scalar_mul(pi, e, rs)
        sb_pi.append(pi)

    # ---- main loop over b-blocks ----
    def all_reduce(src, tag, nheads):
        """AllReduce the RAW per-chunk partials [128, nheads*7]; the 7->1
        per-head reduce happens on DVE after the collective (it is gated
        on the collective anyway, and keeping the DVE queue free of
        pre-collective work lets the collective launch at block end)."""
        n = nheads * 7
        cc_in = dram.tile([128, n], F32, tag=f"ccin{tag}", name=f"ccin{tag}")
        cc_out = dram.tile([128, n], F32, tag=f"ccout{tag}",
                           name=f"ccout{tag}")
        nc.gpsimd.dma_start(out=cc_in[:], in_=src)
        nc.gpsimd.collective_compute(
            "AllReduce", add,
            replica_groups=[list(range(N_CORES))],
            ins=[cc_in.opt()], outs=[cc_out.opt()],
        )
        dst = work.tile([128, nheads, 7], F32, tag=f"sglob{tag}",
                        name=f"sglob{tag}")
        # gpsimd queue, NOT sync: the sync FIFO carries the big output
        # DMAs whose sem-waits would head-of-line-block this read
        nc.gpsimd.dma_start(out=dst, in_=cc_out[:])
        return dst

    def load_weights(i):
        lw = {}
        for h in range(H):
            t = lwp.tile([128, KT, 128], FP8, tag=f"lw{h}", name=f"lw{h}")
            for kc in range(KT):
                nc.sync.dma_start(
                    out=t[:, kc, :],
                    in_=projT_dram[h][kc][:, i * 128:(i + 1) * 128])
            lw[h] = t
        return lw

    def do_mixture(pend):
        """pi-weighted mixture for a completed block; lags production."""
        i, ehalves, s_glob = pend
        s4 = work.tile([128, H], F32, tag="s4", name="s4")
        nc.vector.tensor_reduce(out=s4, in_=s_glob,
                                axis=mybir.AxisListType.X, op=add)
        rS = work.tile([128, H], F32, tag="rS", name="rS")
        nc.vector.reciprocal(rS, s4)
        w = work.tile([128, H], F32, tag="w", name="w")
        nc.vector.tensor_mul(w, sb_pi[i], rS)
        for half, (v0, vw) in enumerate([(0, HALF_A), (HALF_A, HALF_B)]):
            # scale each head in place (4x mode), then add-chain (2x mode)
            # into head 3's tile (allocated last -> freed last, matching
            # the ring's FIFO reuse order). Heads 0..2 are added in THAT
            # order: their tiles are the next ones the ring reuses, so
            # releasing them earliest unblocks the next block's ScalarE.
            eh = [ehalves[(h, half)][:, :vw] for h in range(H)]
            nc.vector.tensor_scalar_mul(eh[3], eh[3], w[:, 3:4])
            for h in range(3):
                nc.vector.tensor_scalar_mul(eh[h], eh[h], w[:, h:h + 1])
                nc.vector.tensor_tensor(out=eh[3], in0=eh[3], in1=eh[h],
                                        op=add)
            nc.sync.dma_start(
                out=out[i * 128:(i + 1) * 128, v0:v0 + vw], in_=eh[3])

    lw_cur = load_weights(0)
    pend = None
    for i in range(N_BBLK):
        # issue the lagged mixture FIRST: the DVE queue is strict FIFO, and
        # this block's production reuses ring slots freed by these reads
        if pend is not None:
            do_mixture(pend)
            pend = None
        sparts = work.tile([128, H * 7], F32, tag="spb", name="spb")
        lw_next = None
        ehalves = {}
        for h in range(H):
            for half, (v0h, chunks) in enumerate(
                    [(0, _CHUNKS_A), (HALF_A, _CHUNKS_B)]):
                ehalf = ering.tile([128, HALF_B], BF16, tag="e",
                                   name=f"e{h}_{half}")
                ehalves[(h, half)] = ehalf
                for ci, (c0, cw) in enumerate(chunks):
                    v0 = v0h + c0
                    ps = psum.tile([128, 2048], F32, tag="ps", name="ps")
                    for ns in range((cw + 511) // 512):
                        n0 = ns * 512
                        nw = min(512, cw - n0)
                        nc.tensor.matmul(
                            ps[:, n0:n0 + nw],
                            lw_cur[h],
                            sb_emb[:, :, v0 + n0:v0 + n0 + nw],
                            start=True, stop=True, perf_mode=DR,
                        )
                    cell = h * 7 + half * 3 + ci
                    nc.scalar.activation(
                        out=ehalf[:, c0:c0 + cw], in_=ps[:, :cw],
                        func=Exp, scale=1.0 / EMB_SCALE,
                        accum_out=sparts[:, cell:cell + 1],
                    )
            if h == 0 and i + 1 < N_BBLK:
                # prefetch next block's weight slices during head 1
                lw_next = load_weights(i + 1)
            if i == N_BBLK - 1 and h == 2:
                # last block: AllReduce heads 0-2 early so their mixture
                # runs while head 3's collective is still in flight
                s_early = all_reduce(sparts[:, 0:21], "012", 3)

        if i < N_BBLK - 1:
            # one AllReduce of all heads' denominator partials per block
            s_glob = all_reduce(sparts, "b", H)
            pend = (i, ehalves, s_glob)
        else:
            s_late = all_reduce(sparts[:, 21:28], "3", 1)
        if lw_next is not None:
            lw_cur = lw_next

    # drain: last block's mixture. Heads 0-2 are scaled and summed into
    # head 0's tiles while head 3's AllReduce is still in flight; only a
    # short TS+TT chain remains after it lands.
    def half_slices(h):
        return [ehalves[(h, 0)][:, :HALF_A], ehalves[(h, 1)][:, :HALF_B]]

    s3 = work.tile([128, 3], F32, tag="s3d", name="s3d")
    nc.vector.tensor_reduce(out=s3, in_=s_early,
                            axis=mybir.AxisListType.X, op=add)
    rS3 = work.tile([128, 3], F32, tag="rS3d", name="rS3d")
    nc.vector.reciprocal(rS3, s3)
    w3 = work.tile([128, 3], F32, tag="w3d", name="w3d")
    nc.vector.tensor_mul(w3, sb_pi[N_BBLK - 1][:, 0:3], rS3)
    acc = half_slices(0)
    for h in range(3):
        for half, eh in enumerate(half_slices(h)):
            nc.vector.tensor_scalar_mul(eh, eh, w3[:, h:h + 1])
            if h > 0:
                nc.vector.tensor_tensor(out=acc[half], in0=acc[half],
                                        in1=eh, op=add)
    s1 = work.tile([128, 1], F32, tag="s1d", name="s1d")
    nc.vector.tensor_reduce(out=s1, in_=s_late,
                            axis=mybir.AxisListType.X, op=add)
    rS1 = work.tile([128, 1], F32, tag="rS1d", name="rS1d")
    nc.vector.reciprocal(rS1, s1)
    w1 = work.tile([128, 1], F32, tag="w1d", name="w1d")
    nc.vector.tensor_mul(w1, sb_pi[N_BBLK - 1][:, 3:4], rS1)
    e3 = half_slices(3)
    for half, (v0, vw) in enumerate([(0, HALF_A), (HALF_A, HALF_B)]):
        nc.vector.tensor_scalar_mul(e3[half], e3[half], w1)
        nc.vector.tensor_tensor(out=acc[half], in0=acc[half], in1=e3[half],
                                op=add)
        nc.sync.dma_start(
            out=out[(N_BBLK - 1) * 128:N_BBLK * 128, v0:v0 + vw],
            in_=acc[half])


def _get_nc():
    if "nc" not in _CACHE:
        _CACHE["nc"] = _build()
    return _CACHE["nc"]


def kernel(x, proj_mat, mix_mat, emb):
    nc = _get_nc()
    bf = ml_dtypes.bfloat16
    f8 = ml_dtypes.float8_e4m3fn
    xT = np.ascontiguousarray(x.astype(bf).T)
    pmT = np.ascontiguousarray(proj_mat.astype(bf).T)
    mmT = np.ascontiguousarray(mix_mat.astype(bf).T)
    emb_s = (emb.astype(np.float32) * EMB_SCALE).astype(f8)
    in_maps = []
    for c in range(N_CORES):
        sub = emb_s[c * V_S:(c + 1) * V_S]          # [V_S, D]
        embT = np.zeros((128, KT, PADV), dtype=f8)
        for j in range(KT):
            embT[:, j, :V_S] = sub[:, j * 128:(j + 1) * 128].T
        in_maps.append({"xT": xT, "pmT": pmT, "mmT": mmT, "embT": embT})
    res = run_bass_kernel_spmd(nc, in_maps, list(range(N_CORES)),
                               **_RUN_KWARGS)
    _CACHE["last_result"] = res
    return np.concatenate(
        [res.results[c]["out"].astype(np.float32) for c in range(N_CORES)],
        axis=1)
